# revision 1
# baseline (speedup 1.0000x reference)
"""Distributed Trainium2 Bass kernel for nn_Attention_50139448213963.

Attention layer with per-head QK-layernorm + interleaved RoPE:
  qkv = x @ Wqkv_w.T + Wqkv_b ; q,k = LN_head(q|k) ; q,k = rope(q|k)
  out = softmax(q k^T / sqrt(d)) v ; out = concat_heads @ out_w.T + out_b

Sharding (8 cores): core c -> batch c//4, heads {2*(c%4), 2*(c%4)+1}
(data parallel on B, tensor parallel on heads).  Each core computes QKV
for its 2 heads, attention, and the out-proj partial using its heads'
columns of out_w.  Host sums the 4 partials per batch; out_b and the
v-bias term (exactly foldable through softmax-normalized attention) are
added host-side.  q/k biases would need the on-device rank-1 path
(emit_qk_bias) -- they are zero for this problem.

Per-core dataflow (matmuls bf16, fp32 PSUM accumulation; modeled
~222 us by TimelineSim, ~68 GFLOP total across 8 cores):
  1. QKV in normal layout [tok, outdim]: lhsT = xT k-chunk (host feeds
     x transposed), rhs = Wqkv slice (host-transposed, q/k head_dim
     PERMUTED even-first so rotate_half becomes a contiguous 64-half
     swap).  One fast ACT eviction frees each PSUM bank at PE rate.
  2. LN via bn_stats/bn_aggr; (x-mu)*rs fused into ScalarE activation
     (scale=rs, bias=-mu*rs).  RoPE: q_gamma/k_gamma and the 1/sqrt(d)
     score scale are folded into host-built cos/sin tables; the
     rotate_half sign is folded into the sin table so the device only
     does two strided multiplies (GPSIMD) + one multiply-add (DVE).
  3. q,k transposed to [d, tok] via PE transposes, software-pipelined
     two tiles back so the in-order PE stream never stalls on them.
  4. Per (head, 512-wide q chunk): scoresT = kT_tile^T @ qT (16 k-tiles,
     no max-subtraction: LN+rope bound |scores| ~ a few sigma), exp on
     ScalarE (PSUM->SBUF bf16), AV split into M=65/M=64 matmuls with a
     ones-column appended to v so row 64 of the first accumulator IS
     the softmax sum (no separate sums pass PSUM bank).
  5. Normalize: reciprocal straight from PSUM -> rank-1 ones matmul
     broadcast -> two half-height multiplies -> avn bf16.
  6. Out-proj per tok tile accumulating both heads, DVE evict, store.

Scheduling: engine sequencers are IN-ORDER, so emission order is the
schedule.  Stage 2 is software-pipelined: block i's scores/AV loop
carries block i-1's normalize (injected mid-loop) and the out-proj of
the chunk completed one block ago.  All PSUM lives in ONE pool with
shared tags (A: qkv|scores|proj 3 bufs, B: v-psum|av1 2, C: tp|av2 2,
Dd: bcast 1 = exactly 8 banks) -- pool open/close would serialize the
stage transition on the allocator's release dependencies.
"""

import math
import os
from contextlib import ExitStack

import numpy as np
import ml_dtypes

import concourse.bass as bass
import concourse.tile as tile
from concourse import bacc, mybir
from concourse.bass import ts, ds
from concourse.bass_utils import run_bass_kernel_spmd
from concourse.masks import make_identity

F32 = mybir.dt.float32
F32R = mybir.dt.float32r
BF16 = mybir.dt.bfloat16

DIM = 1024
HEADS = 8
D = 128  # head dim
B = 2
N = 2048
EPS = 1e-6
HPC = 2  # heads per core
N_CORES = 8
P = 128  # partitions
QC = 512  # q chunk for attention
N_TILES = N // P  # 16
K_IN = DIM // P  # 8 k-tiles over input dim
W_OUT = HPC * 3 * D  # 768 qkv outdims per core
# wqkv block layout (free offsets): q0,k0,q1,k1 then v0,v1
OFF_Q = [0 * D, 2 * D]
OFF_K = [1 * D, 3 * D]
OFF_V = [4 * D, 5 * D]


def build_core_graph(nc, n_tok=N, dtype_mm=BF16, emit_qk_bias=False):
    """Emit the per-core program. All cores run the same graph (SPMD)."""
    n_tiles = n_tok // P
    n_qc = n_tok // QC if n_tok >= QC else 1
    qc = min(QC, n_tok)
    tpq = qc // P  # tok tiles per q chunk

    # ---- dram parameters ----
    xT = nc.dram_tensor("xT", [DIM, n_tok], dtype_mm, kind="ExternalInput").ap()
    wqkv = nc.dram_tensor("wqkv", [DIM, W_OUT], dtype_mm, kind="ExternalInput").ap()
    wout = nc.dram_tensor("wout", [HPC * D, DIM], dtype_mm, kind="ExternalInput").ap()
    ropeq = nc.dram_tensor("ropeq", [n_tok, 2 * HPC * D], BF16, kind="ExternalInput").ap()
    ropek = nc.dram_tensor("ropek", [n_tok, 2 * HPC * D], BF16, kind="ExternalInput").ap()
    bqkv = None
    if emit_qk_bias:
        bqkv = nc.dram_tensor("bqkv", [1, W_OUT], F32, kind="ExternalInput").ap()
    out = nc.dram_tensor("out", [n_tok, DIM], F32, kind="ExternalOutput").ap()

    with tile.TileContext(nc) as tc, ExitStack() as ctx:
        const = ctx.enter_context(tc.tile_pool(name="const", bufs=1))
        big = ctx.enter_context(tc.tile_pool(name="big", bufs=1))

        # resident SBUF tensors
        xT_sb = big.tile([P, K_IN, n_tok], dtype_mm, tag="xT_sb")
        wqkv_sb = big.tile([P, K_IN, W_OUT], dtype_mm, tag="wqkv_sb")
        wout_sb = big.tile([P, HPC, DIM], dtype_mm, tag="wout_sb")
        qT_sb = [big.tile([P, n_tok], dtype_mm, tag=f"qT{h}", name=f"qT{h}")
                 for h in range(HPC)]
        kT_sb = [big.tile([P, n_tok], dtype_mm, tag=f"kT{h}", name=f"kT{h}")
                 for h in range(HPC)]
        # v low halves + ones column (row 64 of AV psum = softmax sums), v high
        v1_all = big.tile([P, n_tiles, HPC, D // 2 + 1], dtype_mm, tag="v1_all")
        v2_all = big.tile([P, n_tiles, HPC, D // 2], dtype_mm, tag="v2_all")
        avn_sb = [big.tile([P, qc], dtype_mm, tag=f"avn{h}", name=f"avn{h}", bufs=min(2, n_qc))
                  for h in range(HPC)]

        ones_row = const.tile([1, P], F32)  # bcast rank-1 lhsT
        nc.vector.memset(ones_row[:], 1.0)
        nc.vector.memset(v1_all[:, :, :, D // 2], 1.0)
        eps_col = const.tile([P, 1], F32)
        nc.vector.memset(eps_col[:], EPS)

        ropeq_sb = big.tile([P, n_tiles, 2 * HPC * D], BF16, tag="ropeq_sb")
        ropek_sb = big.tile([P, n_tiles, 2 * HPC * D], BF16, tag="ropek_sb")

        # load weights/activations
        for kk in range(K_IN):
            half = n_tok // 2
            nc.sync.dma_start(xT_sb[:, kk, 0:half], xT[ts(kk, P), 0:half])
            nc.sync.dma_start(wqkv_sb[:, kk, :], wqkv[ts(kk, P), :])
            nc.sync.dma_start(xT_sb[:, kk, half:n_tok], xT[ts(kk, P), half:n_tok])
        for h in range(HPC):
            nc.sync.dma_start(wout_sb[:, h, :], wout[ts(h, P), :])
        nc.gpsimd.dma_start(ropeq_sb[:],
                            ropeq.rearrange("(t p) f -> p t f", p=P))
        nc.gpsimd.dma_start(ropek_sb[:],
                            ropek.rearrange("(t p) f -> p t f", p=P))

        if emit_qk_bias:
            bias_sb = const.tile([1, W_OUT], F32)
            nc.sync.dma_start(bias_sb[:], bqkv[:])

        # ---------- stage 1: QKV + LN + RoPE ----------
        ident = const.tile([P, P], dtype_mm)
        make_identity(nc, ident)

        # single PSUM pool, tags shared across stages (8 banks, no barriers):
        #   A bufs=3: ps_a | sc | po     B bufs=2: ps_b | av1
        #   C bufs=2: tp | av2           Dd bufs=1: bc
        ps = ctx.enter_context(tc.tile_pool(name="ps", bufs=1, space="PSUM"))
        s1 = ctx.enter_context(tc.tile_pool(name="s1", bufs=6))
        s1small = ctx.enter_context(tc.tile_pool(name="s1small", bufs=8))

        ro_tiles = {}  # (t, "q"/"k") -> rope-applied bf16 tile awaiting transpose

        def emit_transposes(t):
            for name, dst in (("q", qT_sb), ("k", kT_sb)):
                ro = ro_tiles.pop((t, name))
                for h in range(HPC):
                    tp = ps.tile([P, P], BF16, tag="C", bufs=2, name=f"tp{t}{name}{h}")
                    nc.tensor.transpose(tp[:], ro[:, ts(h, D)], ident[:])
                    if name == "q":
                        nc.scalar.activation(dst[h][:, ts(t, P)], tp[:],
                                             mybir.ActivationFunctionType.Copy)
                    else:
                        nc.vector.tensor_copy(dst[h][:, ts(t, P)], tp[:])

        for t in range(n_tiles):
            # two psum chunks: [q0,k0,q1,k1] (512) and [v0,v1] (256)
            ps_a = ps.tile([P, 4 * D], F32, tag="A", bufs=3, name=f"ps_a{t}")
            ps_b = ps.tile([P, 2 * D], F32, tag="B", bufs=2, name=f"ps_b{t}")
            for kk in range(K_IN):
                lhsT = xT_sb[:, kk, ts(t, P)]
                nc.tensor.matmul(ps_a[:], lhsT, wqkv_sb[:, kk, 0:4 * D],
                                 start=(kk == 0), stop=(kk == K_IN - 1))
                nc.tensor.matmul(ps_b[:], lhsT, wqkv_sb[:, kk, 4 * D:W_OUT],
                                 start=(kk == 0), stop=(kk == K_IN - 1))
            if emit_qk_bias:
                nc.tensor.matmul(ps_a[:], ones_row[:, :], bias_sb[:, 0:4 * D],
                                 start=False, stop=True)
                nc.tensor.matmul(ps_b[:], ones_row[:, :], bias_sb[:, 4 * D:W_OUT],
                                 start=False, stop=True)

            # free PSUM fast: one eviction (ACT), v-halves on DVE
            qk_raw = s1.tile([P, 4 * D], F32, tag="qk_raw")
            nc.scalar.activation(qk_raw[:], ps_a[:],
                                 mybir.ActivationFunctionType.Copy)
            pb4 = ps_b.rearrange("p (h x) -> p h x", x=D)
            nc.vector.tensor_copy(v1_all[:, t, :, 0:D // 2], pb4[:, :, 0:D // 2])
            nc.vector.tensor_copy(v2_all[:, t, :, :], pb4[:, :, D // 2:D])

            # LN stats for q0,k0,q1,k1
            stats = s1small.tile([P, 4, 6], F32, tag="stats")
            mv = s1small.tile([P, 4, 2], F32, tag="mv")
            rs = s1small.tile([P, 4], F32, tag="rs")
            for s in range(4):
                nc.vector.bn_stats(stats[:, s, :], qk_raw[:, ts(s, D)])
                nc.vector.bn_aggr(mv[:, s, :], stats[:, s, :])
            # rs = 1/sqrt(var+eps)
            sd = s1small.tile([P, 4], F32, tag="sd")
            nc.scalar.activation(sd[:, :], mv[:, :, 1],
                                 mybir.ActivationFunctionType.Sqrt,
                                 bias=eps_col[:])
            nc.vector.reciprocal(rs[:, :], sd[:, :])

            # bias = -(mu*rs) so ScalarE computes (x*rs + bias) = (x-mu)*rs
            negrs = s1small.tile([P, 4], F32, tag="negrs")
            nc.vector.tensor_scalar(negrs[:, :], rs[:, :], -1.0, None,
                                    mybir.AluOpType.mult)
            negmurs = s1small.tile([P, 4], F32, tag="negmurs")
            nc.vector.tensor_tensor(negmurs[:, :], mv[:, :, 0], negrs[:, :],
                                    mybir.AluOpType.mult)
            qn = s1.tile([P, HPC * D], BF16, tag="qn")
            kn = s1.tile([P, HPC * D], BF16, tag="kn")
            for h in range(HPC):
                nc.scalar.activation(qn[:, ts(h, D)], qk_raw[:, ts(2 * h, D)],
                                     mybir.ActivationFunctionType.Identity,
                                     bias=negmurs[:, 2 * h:2 * h + 1],
                                     scale=rs[:, 2 * h:2 * h + 1])
                nc.scalar.activation(kn[:, ts(h, D)], qk_raw[:, ts(2 * h + 1, D)],
                                     mybir.ActivationFunctionType.Identity,
                                     bias=negmurs[:, 2 * h + 1:2 * h + 2],
                                     scale=rs[:, 2 * h + 1:2 * h + 2])

            # rope: a = qn*cos (DVE); b = halves-swapped qn * sign-folded sin
            # (GPSIMD, sign baked into the table host-side); sum on DVE.
            for (xn, rt, name) in ((qn, ropeq_sb[:, t, :], "q"), (kn, ropek_sb[:, t, :], "k")):
                x4 = xn.rearrange("p (s x) -> p s x", x=D)
                a = s1.tile([P, HPC * D], BF16, tag=f"a{name}")
                bb = s1.tile([P, HPC * D], BF16, tag=f"b{name}")
                b4 = bb.rearrange("p (s x) -> p s x", x=D)
                sinS = rt[:, HPC * D:2 * HPC * D].rearrange("p (s x) -> p s x", x=D)
                nc.vector.tensor_tensor(a[:], xn[:], rt[:, 0:HPC * D],
                                        mybir.AluOpType.mult)
                nc.gpsimd.tensor_tensor(b4[:, :, 0:D // 2], x4[:, :, D // 2:D],
                                        sinS[:, :, 0:D // 2], mybir.AluOpType.mult)
                nc.gpsimd.tensor_tensor(b4[:, :, D // 2:D], x4[:, :, 0:D // 2],
                                        sinS[:, :, D // 2:D], mybir.AluOpType.mult)
                ro = s1.tile([P, HPC * D], dtype_mm, tag=f"ro{name}")
                eng = nc.vector if name == "q" else nc.gpsimd
                eng.tensor_tensor(ro[:], a[:], bb[:], mybir.AluOpType.add)
                ro_tiles[(t, name)] = ro

            # PE transposes, software-pipelined 2 tiles back
            if t >= 2:
                emit_transposes(t - 2)
        emit_transposes(n_tiles - 2)
        emit_transposes(n_tiles - 1)

        # ---------- stage 2+3: attention + out-projection, sw-pipelined ----------
        probs = ctx.enter_context(tc.tile_pool(name="probs", bufs=6))
        s2 = ctx.enter_context(tc.tile_pool(name="s2", bufs=4))
        s3 = ctx.enter_context(tc.tile_pool(name="s3", bufs=6))

        av_tiles = {}

        def emit_block(qi, h, mid=None):
            """scores + exp + fused AV/sums over all k tiles for (qi, h)."""
            av1 = ps.tile([D // 2 + 1, qc], F32, tag="B", bufs=2, name=f"av1_{qi}_{h}")
            av2 = ps.tile([D // 2, qc], F32, tag="C", bufs=2, name=f"av2_{qi}_{h}")
            for kt in range(n_tiles):
                if kt == 6 and mid is not None:
                    mid()
                sc = ps.tile([P, qc], F32, tag="A", bufs=3, name=f"sc{qi}{h}{kt}")
                nc.tensor.matmul(sc[:], kT_sb[h][:, ts(kt, P)],
                                 qT_sb[h][:, ds(qi * qc, qc)],
                                 start=True, stop=True)
                pr = probs.tile([P, qc], dtype_mm, tag="pr")
                nc.scalar.activation(pr[:], sc[:],
                                     mybir.ActivationFunctionType.Exp)
                nc.tensor.matmul(av1[:], v1_all[:, kt, h, :], pr[:],
                                 start=(kt == 0), stop=(kt == n_tiles - 1))
                nc.tensor.matmul(av2[:], v2_all[:, kt, h, :], pr[:],
                                 start=(kt == 0), stop=(kt == n_tiles - 1))
            av_tiles[(qi, h)] = (av1, av2)

        def emit_recip_bc(qi, h):
            """recip of the sums row + rank-1 broadcast, right at block end
            (ahead of proj evictions in DVE program order)."""
            av1, av2 = av_tiles[(qi, h)]
            rcp = s2.tile([1, qc], F32, tag="rcp", name=f"rcp{qi}{h}")
            nc.vector.reciprocal(rcp[:], av1[D // 2:D // 2 + 1, :])
            bc = ps.tile([P, qc], F32, tag="Dd", bufs=1, name=f"bc{qi}{h}")
            nc.tensor.matmul(bc[:], ones_row[:], rcp[:], start=True, stop=True)
            av_tiles[(qi, h)] = (av1, av2, bc)

        def emit_normalize(qi, h):
            """copy broadcast + scale both AV halves -> avn."""
            av1, av2, bc = av_tiles.pop((qi, h))
            bc_sb = s2.tile([P, qc], F32, tag="bc_sb")
            nc.vector.tensor_copy(bc_sb[:], bc[:])
            nc.vector.tensor_tensor(avn_sb[h][0:D // 2, :], av1[0:D // 2, :],
                                    bc_sb[0:D // 2, :], mybir.AluOpType.mult)
            nc.vector.tensor_tensor(avn_sb[h][D // 2:D, :], av2[:, :],
                                    bc_sb[D // 2:D, :], mybir.AluOpType.mult)

        def emit_proj(qi):
            """out-projection + store for this q chunk's tok tiles."""
            for ti in range(tpq):
                t = qi * tpq + ti
                for c in range(DIM // QC):
                    po = ps.tile([P, QC], F32, tag="A", bufs=3, name=f"po{t}{c}")
                    for h in range(HPC):
                        nc.tensor.matmul(po[:], avn_sb[h][:, ts(ti, P)],
                                         wout_sb[:, h, ts(c, QC)],
                                         start=(h == 0), stop=(h == HPC - 1))
                    ot = s3.tile([P, QC], F32, tag="ot")
                    nc.vector.tensor_copy(ot[:], po[:])
                    nc.scalar.dma_start(out[ts(t, P), ts(c, QC)], ot[:])

        blocks = [(qi, h) for qi in range(n_qc) for h in range(HPC)]
        for i, (qi, h) in enumerate(blocks):
            mid = (lambda p=blocks[i - 1]: emit_normalize(*p)) if i >= 1 else None
            emit_block(qi, h, mid=mid)
            emit_recip_bc(qi, h)
            if i >= 1 and blocks[i - 1][1] == HPC - 1:
                emit_proj(blocks[i - 1][0])
        emit_normalize(*blocks[-1])
        emit_proj(blocks[-1][0])

    return nc


# ---------------- host side ----------------

def _prep_core_inputs(x, Wqkv_w, Wqkv_b, q_gamma, k_gamma, out_w,
                      rope_cos, rope_sin, n_tok=N):
    """Build the 8 per-core input dicts (numpy, host-side sharding)."""
    bf = ml_dtypes.bfloat16
    scale = 1.0 / math.sqrt(D)
    # even-first permutation of head_dim and the rope partner map
    perm = np.concatenate([np.arange(0, D, 2), np.arange(1, D, 2)])
    partner = np.concatenate([np.arange(0, D, 2) + 1, np.arange(1, D, 2) - 1])
    # tables in permuted space; gamma folded in; q side also gets 1/sqrt(d)
    cosP = rope_cos[:, perm]
    sinP = rope_sin[:, perm]
    gq, gk = q_gamma, k_gamma
    cos_q = (cosP * gq[perm][None, :]) * scale
    sin_q = (sinP * gq[partner][None, :]) * scale
    cos_k = cosP * gk[perm][None, :]
    sin_k = sinP * gk[partner][None, :]
    # per-tile tables hold both heads side by side: [cos|cos|sin|sin]
    # fold rotate-half's sign into the sin tables: b[j<64] = q[j+64]*(-sin[j]),
    # b[j>=64] = q[j-64]*(+sin[j]) -- the device then only swaps halves via APs
    sgn = np.concatenate([-np.ones(D // 2, np.float32), np.ones(D // 2, np.float32)])
    sin_qS = sin_q * sgn[None, :]
    sin_kS = sin_k * sgn[None, :]
    ropeq = np.concatenate([cos_q, cos_q, sin_qS, sin_qS], axis=1).astype(bf)
    ropek = np.concatenate([cos_k, cos_k, sin_kS, sin_kS], axis=1).astype(bf)

    Wr = Wqkv_w.reshape(3, HEADS, D, DIM)
    in_maps = []
    for c in range(N_CORES):
        b = c // 4
        hs = [2 * (c % 4), 2 * (c % 4) + 1]
        xT = np.ascontiguousarray(x[b, :n_tok].T).astype(bf)
        blocks = []
        for h in hs:
            blocks.append(Wr[0, h][perm].T)  # q, dim-permuted  [DIM,128]
            blocks.append(Wr[1, h][perm].T)  # k, dim-permuted
        for h in hs:
            blocks.append(Wr[2, h].T)        # v, natural
        wqkv = np.concatenate(blocks, axis=1).astype(bf)  # [DIM, 768]
        wout = np.concatenate(
            [out_w[:, h * D:(h + 1) * D].T for h in hs], axis=0).astype(bf)  # [256,DIM]
        in_maps.append({
            "xT": xT,
            "wqkv": np.ascontiguousarray(wqkv),
            "wout": np.ascontiguousarray(wout),
            "ropeq": ropeq[:n_tok],
            "ropek": ropek[:n_tok],
        })
    return in_maps


def kernel(x, Wqkv_w, Wqkv_b, q_gamma, q_beta, k_gamma, k_beta,
           out_w, out_b, rope_cos, rope_sin, trace=False, tmpdir=None):
    x = np.asarray(x, np.float32)
    Wqkv_w = np.asarray(Wqkv_w, np.float32)
    Wqkv_b = np.asarray(Wqkv_b, np.float32)
    q_gamma = np.asarray(q_gamma, np.float32)
    q_beta = np.asarray(q_beta, np.float32)
    k_gamma = np.asarray(k_gamma, np.float32)
    k_beta = np.asarray(k_beta, np.float32)
    out_w = np.asarray(out_w, np.float32)
    out_b = np.asarray(out_b, np.float32)
    rope_cos = np.asarray(rope_cos, np.float32)
    rope_sin = np.asarray(rope_sin, np.float32)

    assert np.allclose(q_beta, 0) and np.allclose(k_beta, 0), \
        "nonzero q/k layernorm beta not supported by this kernel build"
    emit_qk_bias = not (np.allclose(Wqkv_b[:DIM], 0) and np.allclose(Wqkv_b[DIM:2 * DIM], 0))

    nc = bacc.Bacc("TRN2", target_bir_lowering=False, debug=False,
                   num_devices=N_CORES)
    build_core_graph(nc, n_tok=N, emit_qk_bias=emit_qk_bias)
    nc.compile()

    in_maps = _prep_core_inputs(x, Wqkv_w, Wqkv_b, q_gamma, k_gamma,
                                out_w, rope_cos, rope_sin)
    if emit_qk_bias:
        for c in range(N_CORES):
            hs = [2 * (c % 4), 2 * (c % 4) + 1]
            bq = Wqkv_b[:DIM].reshape(HEADS, D)
            bk = Wqkv_b[DIM:2 * DIM].reshape(HEADS, D)
            perm = np.concatenate([np.arange(0, D, 2), np.arange(1, D, 2)])
            blocks = [np.zeros(0, np.float32)]
            for h in hs:
                blocks += [bq[h][perm], bk[h][perm]]
            blocks += [np.zeros(2 * D, np.float32)]
            in_maps[c]["bqkv"] = np.concatenate(blocks)[None, :].astype(np.float32)

    res = run_bass_kernel_spmd(nc, in_maps, core_ids=list(range(N_CORES)),
                               trace=trace, tmpdir=tmpdir)
    partials = [np.asarray(r["out"], np.float32) for r in res.results]

    # host gather: sum the 4 head-group partials per batch; fold v-bias + out_b
    bv = Wqkv_b[2 * DIM:]
    bias_row = out_b + bv @ out_w.T  # [DIM]
    outp = np.empty((B, N, DIM), np.float32)
    for b in range(B):
        outp[b] = sum(partials[4 * b:4 * b + 4]) + bias_row[None, :]
    kernel.last_exec_time_ns = res.exec_time_ns
    return outp



# revision 28
# speedup vs baseline: 254.5463x; 254.5463x over previous
"""Distributed Trainium2 Bass kernel for nn_Attention_50139448213963.

Attention layer with per-head QK-layernorm + interleaved RoPE:
  qkv = x @ Wqkv_w.T + Wqkv_b ; q,k = LN_head(q|k) ; q,k = rope(q|k)
  out = softmax(q k^T / sqrt(d)) v ; out = concat_heads @ out_w.T + out_b

Sharding (8 cores): core c -> batch c//4, heads {2*(c%4), 2*(c%4)+1}
(data parallel on B, tensor parallel on heads).  Each core computes QKV
for its 2 heads, attention, and the out-proj partial using its heads'
columns of out_w.  Host sums the 4 partials per batch; out_b and the
v-bias term (exactly foldable through softmax-normalized attention) are
added host-side.  q/k biases would need the on-device rank-1 path
(emit_qk_bias) -- they are zero for this problem.

Per-core dataflow (matmuls bf16, fp32 PSUM accumulation):
  1. Input DMA on ONE queue in dependency order: [wqkv kk | xT kk blk0]
     interleaved so the first QKV matmul can start after ~220KB, then
     rope blk0, xT blk1, rope blk1, ... wout last.  Rope tables carry no
     head duplication and are SHARED between q and k when the gammas
     coincide (they are both ones here); the 1/sqrt(d) score scale is
     applied by the exp activation instead of being folded into cos/sin.
  2. QKV in normal layout [tok, outdim]; LN stats (bn_stats/bn_aggr) and
     the LN apply (ScalarE activation, scale=rs bias=-mu*rs) both read
     the PSUM accumulator directly -- no staging eviction.  RoPE: a-term
     cos multiply + final adds on DVE, halves-swapped sign-folded
     sin-multiplies on GPSIMD.
  3. q,k transposed to [d, tok] via PE transposes, software-pipelined
     two tiles back so the in-order PE stream never stalls on them.
  4. Per (head, 512-wide q chunk): scoresT = kT_tile^T @ qT (16 k-tiles,
     no max-subtraction: LN+rope bound |scores| ~ a few sigma), exp on
     ScalarE (PSUM->SBUF bf16, scale=1/sqrt(d)), AV split into M=65/M=64
     matmuls with a ones-column appended to v so row 64 of the first
     accumulator IS the softmax sum (no separate sums pass).
  5. Normalize: reciprocal straight from PSUM -> rank-1 ones matmul
     broadcast -> two half-height multiplies -> avn bf16.
  6. Out-proj per tok tile accumulating both heads, DVE evict to fp16,
     store (fp16 halves the store DMA; host upcasts and sums partials).

Scheduling: engine sequencers are IN-ORDER, so emission order is the
schedule.  Stage 2 is software-pipelined: block i's scores/AV loop
carries block i-1's normalize (injected mid-loop) and the out-proj of
the chunk completed one block ago.  All PSUM lives in ONE pool with
shared tags (A: qkv|scores|proj 3 bufs, B: v-psum|av1 2, C: tp|av2 2,
Dd: bcast 1 = exactly 8 banks) -- pool open/close would serialize the
stage transition on the allocator's release dependencies.

`repeat=k` wraps the body in a tc.For_i hardware loop so one NEFF
executes the whole kernel k times back-to-back; test.py uses the
slope between two repeat counts to measure true on-device time
through the high-latency PJRT tunnel.
"""

import math
import os
from contextlib import ExitStack

import numpy as np
import ml_dtypes

import concourse.bass as bass
import concourse.tile as tile
from concourse import bacc, mybir
from concourse.bass import ts, ds
from concourse.bass_utils import run_bass_kernel_spmd
from concourse.masks import make_identity

F32 = mybir.dt.float32
F16 = mybir.dt.float16
BF16 = mybir.dt.bfloat16

DIM = 1024
HEADS = 8
D = 128  # head dim
B = 2
N = 2048
EPS = 1e-6
HPC = 2  # heads per core
N_CORES = 8
P = 128  # partitions
QC = 512  # q chunk for attention
N_TILES = N // P  # 16
K_IN = DIM // P  # 8 k-tiles over input dim
W_OUT = HPC * 3 * D  # 768 qkv outdims per core
SCALE = 1.0 / math.sqrt(D)
# wqkv block layout (free offsets): q0,k0,q1,k1 then v0,v1
OFF_Q = [0 * D, 2 * D]
OFF_K = [1 * D, 3 * D]
OFF_V = [4 * D, 5 * D]


def build_core_graph(nc, n_tok=N, dtype_mm=BF16, emit_qk_bias=False,
                     shared_rope=True, repeat=1):
    """Emit the per-core program. All cores run the same graph (SPMD)."""
    n_tiles = n_tok // P
    n_qc = n_tok // QC if n_tok >= QC else 1
    qc = min(QC, n_tok)
    tpq = qc // P  # tok tiles per q chunk
    n_blk = min(4, n_tiles)  # token blocks for the load pipeline
    tpb = n_tiles // n_blk   # tiles per block
    RW = 2 * D if shared_rope else 4 * D  # rope row width (cos|sin[|cos|sin])

    # ---- dram parameters ----
    xT = nc.dram_tensor("xT", [DIM, n_tok], dtype_mm, kind="ExternalInput").ap()
    wqkv = nc.dram_tensor("wqkv", [DIM, W_OUT], dtype_mm, kind="ExternalInput").ap()
    wout = nc.dram_tensor("wout", [HPC * D, DIM], dtype_mm, kind="ExternalInput").ap()
    rope = nc.dram_tensor("rope", [n_tok, RW], BF16, kind="ExternalInput").ap()
    bqkv = None
    if emit_qk_bias:
        bqkv = nc.dram_tensor("bqkv", [1, W_OUT], F32, kind="ExternalInput").ap()
    out = nc.dram_tensor("out", [n_tok, DIM], F16, kind="ExternalOutput").ap()

    with tile.TileContext(nc) as tc, ExitStack() as ctx:
        const = ctx.enter_context(tc.tile_pool(name="const", bufs=1))
        big = ctx.enter_context(tc.tile_pool(name="big", bufs=1))

        # resident SBUF tensors
        xT_sb = big.tile([P, K_IN, n_tok], dtype_mm, tag="xT_sb")
        wqkv_sb = big.tile([P, K_IN, W_OUT], dtype_mm, tag="wqkv_sb")
        wout_sb = big.tile([P, HPC, DIM], dtype_mm, tag="wout_sb")
        rope_sb = big.tile([P, n_tiles, RW], BF16, tag="rope_sb")
        qT_all = big.tile([P, HPC, n_tok], dtype_mm, tag="qT_all")
        kT_all = big.tile([P, HPC, n_tok], dtype_mm, tag="kT_all")
        # v + trailing ones column: av1 takes cols [64:129] (v hi-half + ones
        # -> its psum row 64 IS the softmax sum), av2 takes cols [0:64]
        v_all = big.tile([P, n_tiles, HPC, D + 1], dtype_mm, tag="v_all")
        avn_sb = [big.tile([P, qc], dtype_mm, tag=f"avn{h}", name=f"avn{h}", bufs=min(2, n_qc))
                  for h in range(HPC)]

        ones_row = const.tile([1, P], F32)  # bcast rank-1 lhsT
        eps_col = const.tile([P, 1], F32)
        ident = const.tile([P, P], dtype_mm)
        bias_sb = const.tile([1, W_OUT], F32) if emit_qk_bias else None

        # single PSUM pool, tags shared across stages (8 banks, no barriers):
        #   A bufs=4: ps_a | sc | po     B bufs=2: ps_b | av1
        #   C bufs=2: tp | av2
        ps = ctx.enter_context(tc.tile_pool(name="ps", bufs=1, space="PSUM"))
        s1 = ctx.enter_context(tc.tile_pool(name="s1", bufs=6))
        s1small = ctx.enter_context(tc.tile_pool(name="s1small", bufs=8))
        probs = ctx.enter_context(tc.tile_pool(name="probs", bufs=6))
        s2 = ctx.enter_context(tc.tile_pool(name="s2", bufs=4))
        s3 = ctx.enter_context(tc.tile_pool(name="s3", bufs=8))

        def body():
            nc.vector.memset(ones_row[:], 1.0)
            nc.vector.memset(v_all[:, :, :, D], 1.0)
            nc.vector.memset(eps_col[:], EPS)
            make_identity(nc, ident)

            # ---- input loads: ONE queue, few big DMAs, dependency order ----
            # (HWDGE descriptor generation is ~600ns/DMA: batching transfers
            # matters as much as ordering them.)
            xT_r = xT.rearrange("(k p) n -> p k n", p=P)      # [P, K_IN, n_tok]
            rope_r = rope.rearrange("(t p) f -> p t f", p=P)  # [P, n_tiles, RW]
            # block 0 fine-grained so the first QKV matmul starts after ~220KB
            for kk in range(K_IN):
                nc.sync.dma_start(wqkv_sb[:, kk, :], wqkv[ts(kk, P), :])
                nc.sync.dma_start(xT_sb[:, kk, 0:tpb * P], xT[ts(kk, P), 0:tpb * P])
            if emit_qk_bias:
                nc.sync.dma_start(bias_sb[:], bqkv[:])
            nc.sync.dma_start(rope_sb[:, 0:tpb, :], rope_r[:, 0:tpb, :])
            for b in range(1, n_blk):
                tsl = ds(b * tpb * P, tpb * P)
                nc.sync.dma_start(xT_sb[:, :, tsl], xT_r[:, :, tsl])
                nc.sync.dma_start(rope_sb[:, b * tpb:(b + 1) * tpb, :],
                                  rope_r[:, b * tpb:(b + 1) * tpb, :])
            nc.sync.dma_start(wout_sb[:], wout.rearrange("(h p) w -> p h w", p=P))

            # ---------- stage 1: QKV + LN + RoPE ----------
            ro_tiles = {}  # (t, "q"/"k") -> rope-applied bf16 tile awaiting transpose

            def emit_transposes(t):
                ro = ro_tiles.pop(t)  # [P, 2(qk), HPC*D]
                for gi, dst in ((0, qT_all), (1, kT_all)):
                    for h in range(HPC):
                        tp = ps.tile([P, P], BF16, tag="C", bufs=2,
                                     name=f"tp{t}{gi}{h}")
                        nc.tensor.transpose(tp[:], ro[:, gi, ts(h, D)], ident[:])
                        # spread the psum->sbuf copies: q on ACT, k on DVE
                        if gi == 0:
                            nc.scalar.activation(dst[:, h, ts(t, P)], tp[:],
                                                 mybir.ActivationFunctionType.Copy)
                        else:
                            nc.vector.tensor_copy(dst[:, h, ts(t, P)], tp[:])

            for t in range(n_tiles):
                # two psum chunks: [q0,k0,q1,k1] (512) and [v0,v1] (256)
                ps_a = ps.tile([P, 4 * D], F32, tag="A", bufs=4, name=f"ps_a{t}")
                ps_b = ps.tile([P, 2 * D], F32, tag="B", bufs=2, name=f"ps_b{t}")
                for kk in range(K_IN):
                    lhsT = xT_sb[:, kk, ts(t, P)]
                    nc.tensor.matmul(ps_a[:], lhsT, wqkv_sb[:, kk, 0:4 * D],
                                     start=(kk == 0), stop=(kk == K_IN - 1))
                    nc.tensor.matmul(ps_b[:], lhsT, wqkv_sb[:, kk, 4 * D:W_OUT],
                                     start=(kk == 0), stop=(kk == K_IN - 1))
                if emit_qk_bias:
                    nc.tensor.matmul(ps_a[:], ones_row[:, :], bias_sb[:, 0:4 * D],
                                     start=False, stop=True)
                    nc.tensor.matmul(ps_b[:], ones_row[:, :], bias_sb[:, 4 * D:W_OUT],
                                     start=False, stop=True)

                # LN stats straight from PSUM (DVE).  The HW verifier requires
                # exactly one 6-element group per BNStats instruction.
                stats = s1small.tile([P, 4, 6], F32, tag="stats")
                mv = s1small.tile([P, 4, 2], F32, tag="mv")
                rs = s1small.tile([P, 4], F32, tag="rs")
                pa4 = ps_a.rearrange("p (s x) -> p s x", x=D)
                for s in range(4):
                    nc.vector.bn_stats(stats[:, s, :], pa4[:, s, :])
                    nc.vector.bn_aggr(mv[:, s, :], stats[:, s, :])
                # v evicted in one strided copy (ones column at D untouched)
                nc.scalar.activation(
                    v_all[:, t, :, 0:D],
                    ps_b.rearrange("p (h x) -> p h x", x=D),
                    mybir.ActivationFunctionType.Copy)

                # rs = 1/sqrt(var+eps)
                sd = s1small.tile([P, 4], F32, tag="sd")
                nc.scalar.activation(sd[:, :], mv[:, :, 1],
                                     mybir.ActivationFunctionType.Sqrt,
                                     bias=eps_col[:])
                nc.vector.reciprocal(rs[:, :], sd[:, :])

                # bias = -(mu*rs) so ScalarE computes (x*rs + bias) = (x-mu)*rs
                negrs = s1small.tile([P, 4], F32, tag="negrs")
                nc.gpsimd.tensor_scalar(negrs[:, :], rs[:, :], -1.0, None,
                                        mybir.AluOpType.mult)
                negmurs = s1small.tile([P, 4], F32, tag="negmurs")
                nc.gpsimd.tensor_tensor(negmurs[:, :], mv[:, :, 0], negrs[:, :],
                                        mybir.AluOpType.mult)
                # LN apply straight from PSUM (ACT) into one [P, 2(qk), 2h*D]
                # tile so rope processes q and k in single wide ops
                qkn = s1.tile([P, 2, HPC * D], BF16, tag="qkn")
                for h in range(HPC):
                    nc.scalar.activation(qkn[:, 0, ts(h, D)], ps_a[:, ts(2 * h, D)],
                                         mybir.ActivationFunctionType.Identity,
                                         bias=negmurs[:, 2 * h:2 * h + 1],
                                         scale=rs[:, 2 * h:2 * h + 1])
                    nc.scalar.activation(qkn[:, 1, ts(h, D)], ps_a[:, ts(2 * h + 1, D)],
                                         mybir.ActivationFunctionType.Identity,
                                         bias=negmurs[:, 2 * h + 1:2 * h + 2],
                                         scale=rs[:, 2 * h + 1:2 * h + 2])

                # rope over q and k at once: x4 [P, 2(qk), 2(h), D]; tables
                # broadcast over the head dim (and the qk dim when shared).
                # a = x*cos (DVE), b = halves-swapped x * sign-folded sin
                # (Pool), ro = a + b (DVE).
                x4 = qkn.rearrange("p g (s x) -> p g s x", x=D)
                if shared_rope:
                    ct = rope_sb[:, t, :].rearrange("p (o u x) -> p o u x", o=1, u=1)
                    cosT, sinT = ct[:, :, :, 0:D], ct[:, :, :, D:2 * D]
                else:
                    ct = rope_sb[:, t, :].rearrange("p (g u x) -> p g u x", g=2, u=1)
                    cosT, sinT = ct[:, :, :, 0:D], ct[:, :, :, D:2 * D]
                a = s1.tile([P, 2, HPC * D], BF16, tag="ra")
                a4 = a.rearrange("p g (s x) -> p g s x", x=D)
                bb = s1.tile([P, 2, HPC * D], BF16, tag="rb")
                b4 = bb.rearrange("p g (s x) -> p g s x", x=D)
                i0, i1 = bass.broadcast_tensor_aps(x4[:], cosT)
                nc.vector.tensor_tensor(a4[:], i0, i1, mybir.AluOpType.mult)
                j0, j1 = bass.broadcast_tensor_aps(x4[:, :, :, D // 2:D],
                                                   sinT[:, :, :, 0:D // 2])
                nc.gpsimd.tensor_tensor(b4[:, :, :, 0:D // 2], j0, j1,
                                        mybir.AluOpType.mult)
                j0, j1 = bass.broadcast_tensor_aps(x4[:, :, :, 0:D // 2],
                                                   sinT[:, :, :, D // 2:D])
                nc.gpsimd.tensor_tensor(b4[:, :, :, D // 2:D], j0, j1,
                                        mybir.AluOpType.mult)
                ro = s1.tile([P, 2, HPC * D], dtype_mm, tag="ro")
                nc.vector.tensor_tensor(ro[:], a[:], bb[:], mybir.AluOpType.add)
                ro_tiles[t] = ro

                # PE transposes, software-pipelined 3 tiles back
                if t >= 3:
                    emit_transposes(t - 3)
            for tt in range(max(0, n_tiles - 3), n_tiles):
                emit_transposes(tt)

            # ---------- stage 2+3: attention + out-projection, sw-pipelined ----------
            av_tiles = {}

            def emit_block(qi, h, mid=None, work=()):
                """scores + exp + fused AV/sums over all k tiles for (qi, h).
                `mid` fires once mid-loop; `work` items (prev chunk's proj
                pieces) are spread one per k tile from kt=2 on, so their
                psum-bank grabs interleave with the score tiles'."""
                av1 = ps.tile([D // 2 + 1, qc], F32, tag="B", bufs=2,
                              name=f"av1_{qi}_{h}")
                av2 = ps.tile([D // 2, qc], F32, tag="C", bufs=2,
                              name=f"av2_{qi}_{h}")
                # `work` items read the avn generation that `mid` (the next
                # normalize) overwrites -- every item MUST be emitted before
                # mid() or it binds to the wrong generation.
                mid_kt = min(12, n_tiles - 1)
                work = list(work)
                for kt in range(n_tiles):
                    if kt == mid_kt:
                        for w in work:
                            w()
                        work = []
                        if mid is not None:
                            mid()
                    if kt >= 2 and work:
                        work.pop(0)()
                    sc = ps.tile([P, qc], F32, tag="A", bufs=4, name=f"sc{qi}{h}{kt}")
                    nc.tensor.matmul(sc[:], kT_all[:, h, ts(kt, P)],
                                     qT_all[:, h, ds(qi * qc, qc)],
                                     start=True, stop=True)
                    pr = probs.tile([P, qc], dtype_mm, tag="pr")
                    nc.scalar.activation(pr[:], sc[:],
                                         mybir.ActivationFunctionType.Exp,
                                         scale=SCALE)
                    nc.tensor.matmul(av1[:], v_all[:, kt, h, D // 2:D + 1], pr[:],
                                     start=(kt == 0), stop=(kt == n_tiles - 1))
                    nc.tensor.matmul(av2[:], v_all[:, kt, h, 0:D // 2], pr[:],
                                     start=(kt == 0), stop=(kt == n_tiles - 1))
                av_tiles[(qi, h)] = (av1, av2)

            def emit_recip_bc(qi, h):
                """recip of the sums row (av1 row 64) + partition broadcast on
                GPSIMD (idle in this stage) -> bc_sb, no psum bank needed."""
                av1, av2 = av_tiles[(qi, h)]
                rcp = s2.tile([1, qc], F32, tag="rcp", name=f"rcp{qi}{h}")
                nc.vector.reciprocal(rcp[:], av1[D // 2:D // 2 + 1, :])
                bc_sb = s2.tile([P, qc], F32, tag="bc_sb", name=f"bc{qi}{h}")
                nc.gpsimd.partition_broadcast(bc_sb[:], rcp[:])
                av_tiles[(qi, h)] = (av1, av2, bc_sb)

            def emit_normalize(qi, h):
                """scale both AV halves -> avn (av1 rows 0:64 are v dims
                64:128, av2 rows are v dims 0:64)."""
                av1, av2, bc_sb = av_tiles.pop((qi, h))
                nc.vector.tensor_tensor(avn_sb[h][0:D // 2, :], av2[:, :],
                                        bc_sb[0:D // 2, :], mybir.AluOpType.mult)
                nc.vector.tensor_tensor(avn_sb[h][D // 2:D, :], av1[0:D // 2, :],
                                        bc_sb[D // 2:D, :], mybir.AluOpType.mult)

            def proj_chunk(qi, ti, c):
                t = qi * tpq + ti
                po = ps.tile([P, QC], F32, tag="A", bufs=4, name=f"po{t}{c}")
                for h in range(HPC):
                    nc.tensor.matmul(po[:], avn_sb[h][:, ts(ti, P)],
                                     wout_sb[:, h, ts(c, QC)],
                                     start=(h == 0), stop=(h == HPC - 1))
                ot = s3.tile([P, QC], F16, tag="ot")
                nc.vector.tensor_copy(ot[:], po[:])
                nc.scalar.dma_start(out[ts(t, P), ts(c, QC)], ot[:])

            def proj_work(qi):
                return [lambda ti=ti, c=c: proj_chunk(qi, ti, c)
                        for ti in range(tpq) for c in range(DIM // QC)]

            blocks = [(qi, h) for qi in range(n_qc) for h in range(HPC)]
            for i, (qi, h) in enumerate(blocks):
                mid = (lambda p=blocks[i - 1]: emit_normalize(*p)) if i >= 1 else None
                # proj of chunk qi-1 rides inside block (qi, 1): its avn was
                # completed by the normalize injected into block (qi, 0)
                work = proj_work(qi - 1) if (h == HPC - 1 and qi >= 1) else ()
                emit_block(qi, h, mid=mid, work=work)
                emit_recip_bc(qi, h)
            emit_normalize(*blocks[-1])
            for w in proj_work(n_qc - 1):
                w()

        if repeat == 1:
            body()
        else:
            with tc.For_i(0, repeat, 1):
                body()

    return nc


# ---------------- host side ----------------

def _rope_tables(q_gamma, k_gamma, rope_cos, rope_sin, n_tok=N):
    """Build the on-device rope table(s): [cos|sin] rows, sign of the
    rotate-half folded into sin, gamma folded in, NO head duplication and
    NO score scale (the exp activation applies 1/sqrt(d)).  Returns
    (table [n_tok, 2D or 4D], shared: bool)."""
    bf = ml_dtypes.bfloat16
    perm = np.concatenate([np.arange(0, D, 2), np.arange(1, D, 2)])
    partner = np.concatenate([np.arange(0, D, 2) + 1, np.arange(1, D, 2) - 1])
    sgn = np.concatenate([-np.ones(D // 2, np.float32), np.ones(D // 2, np.float32)])
    cosP = rope_cos[:n_tok, perm]
    sinP = rope_sin[:n_tok, perm]

    def tab(g):
        c = cosP * g[perm][None, :]
        s = (sinP * g[partner][None, :]) * sgn[None, :]
        return np.concatenate([c, s], axis=1)

    shared = bool(np.allclose(q_gamma, k_gamma))
    if shared:
        t = tab(q_gamma)
    else:
        t = np.concatenate([tab(q_gamma), tab(k_gamma)], axis=1)
    return np.ascontiguousarray(t.astype(bf)), shared


def _prep_core_inputs(x, Wqkv_w, Wqkv_b, q_gamma, k_gamma, out_w,
                      rope_cos, rope_sin, n_tok=N):
    """Build the 8 per-core input dicts (numpy, host-side sharding)."""
    bf = ml_dtypes.bfloat16
    # even-first permutation of head_dim (rotate_half becomes a 64-half swap)
    perm = np.concatenate([np.arange(0, D, 2), np.arange(1, D, 2)])
    rope_tab, shared = _rope_tables(q_gamma, k_gamma, rope_cos, rope_sin, n_tok)

    Wr = Wqkv_w.reshape(3, HEADS, D, DIM)
    in_maps = []
    for c in range(N_CORES):
        b = c // 4
        hs = [2 * (c % 4), 2 * (c % 4) + 1]
        xT = np.ascontiguousarray(x[b, :n_tok].T).astype(bf)
        blocks = []
        for h in hs:
            blocks.append(Wr[0, h][perm].T)  # q, dim-permuted  [DIM,128]
            blocks.append(Wr[1, h][perm].T)  # k, dim-permuted
        for h in hs:
            blocks.append(Wr[2, h].T)        # v, natural
        wqkv = np.concatenate(blocks, axis=1).astype(bf)  # [DIM, 768]
        wout = np.concatenate(
            [out_w[:, h * D:(h + 1) * D].T for h in hs], axis=0).astype(bf)  # [256,DIM]
        in_maps.append({
            "xT": xT,
            "wqkv": np.ascontiguousarray(wqkv),
            "wout": np.ascontiguousarray(wout),
            "rope": rope_tab,
        })
    return in_maps, shared


def kernel(x, Wqkv_w, Wqkv_b, q_gamma, q_beta, k_gamma, k_beta,
           out_w, out_b, rope_cos, rope_sin, trace=False, tmpdir=None):
    x = np.asarray(x, np.float32)
    Wqkv_w = np.asarray(Wqkv_w, np.float32)
    Wqkv_b = np.asarray(Wqkv_b, np.float32)
    q_gamma = np.asarray(q_gamma, np.float32)
    q_beta = np.asarray(q_beta, np.float32)
    k_gamma = np.asarray(k_gamma, np.float32)
    k_beta = np.asarray(k_beta, np.float32)
    out_w = np.asarray(out_w, np.float32)
    out_b = np.asarray(out_b, np.float32)
    rope_cos = np.asarray(rope_cos, np.float32)
    rope_sin = np.asarray(rope_sin, np.float32)

    assert np.allclose(q_beta, 0) and np.allclose(k_beta, 0), \
        "nonzero q/k layernorm beta not supported by this kernel build"
    emit_qk_bias = not (np.allclose(Wqkv_b[:DIM], 0) and np.allclose(Wqkv_b[DIM:2 * DIM], 0))

    in_maps, shared = _prep_core_inputs(x, Wqkv_w, Wqkv_b, q_gamma, k_gamma,
                                        out_w, rope_cos, rope_sin)

    nc = bacc.Bacc("TRN2", target_bir_lowering=False, debug=False,
                   num_devices=N_CORES)
    build_core_graph(nc, n_tok=N, emit_qk_bias=emit_qk_bias, shared_rope=shared)
    nc.compile()

    if emit_qk_bias:
        for c in range(N_CORES):
            hs = [2 * (c % 4), 2 * (c % 4) + 1]
            bq = Wqkv_b[:DIM].reshape(HEADS, D)
            bk = Wqkv_b[DIM:2 * DIM].reshape(HEADS, D)
            perm = np.concatenate([np.arange(0, D, 2), np.arange(1, D, 2)])
            blocks = [np.zeros(0, np.float32)]
            for h in hs:
                blocks += [bq[h][perm], bk[h][perm]]
            blocks += [np.zeros(2 * D, np.float32)]
            in_maps[c]["bqkv"] = np.concatenate(blocks)[None, :].astype(np.float32)

    res = run_bass_kernel_spmd(nc, in_maps, core_ids=list(range(N_CORES)),
                               trace=trace, tmpdir=tmpdir)
    partials = [np.asarray(r["out"], np.float32) for r in res.results]

    # host gather: sum the 4 head-group partials per batch; fold v-bias + out_b
    bv = Wqkv_b[2 * DIM:]
    bias_row = out_b + bv @ out_w.T  # [DIM]
    outp = np.empty((B, N, DIM), np.float32)
    for b in range(B):
        outp[b] = sum(partials[4 * b:4 * b + 4]) + bias_row[None, :]
    kernel.last_exec_time_ns = res.exec_time_ns
    return outp


# revision 29
# speedup vs baseline: 413.4182x; 1.6241x over previous
"""Distributed Trainium2 Bass kernel for nn_Attention_50139448213963.

Attention layer with per-head QK-layernorm + interleaved RoPE:
  qkv = x @ Wqkv_w.T + Wqkv_b ; q,k = LN_head(q|k) ; q,k = rope(q|k)
  out = softmax(q k^T / sqrt(d)) v ; out = concat_heads @ out_w.T + out_b

Sharding (8 cores): core c -> batch c//4, heads {2*(c%4), 2*(c%4)+1}
(data parallel on B, tensor parallel on heads).  Each core computes QKV
for its 2 heads, attention, and the out-proj partial using its heads'
columns of out_w.  Host sums the 4 partials per batch; out_b and the
v-bias term (exactly foldable through softmax-normalized attention) are
added host-side.  q/k biases would need the on-device rank-1 path
(emit_qk_bias) -- they are zero for this problem.

Per-core dataflow (matmuls bf16, fp32 PSUM accumulation):
  1. Input DMA on ONE queue in dependency order: [wqkv kk | xT kk blk0]
     interleaved so the first QKV matmul can start after ~220KB, then
     rope blk0, xT blk1, rope blk1, ... wout last.  Rope tables carry no
     head duplication and are SHARED between q and k when the gammas
     coincide (they are both ones here); the 1/sqrt(d) score scale is
     applied by the exp activation instead of being folded into cos/sin.
  2. QKV in normal layout [tok, outdim]; LN stats (bn_stats/bn_aggr) and
     the LN apply (ScalarE activation, scale=rs bias=-mu*rs) both read
     the PSUM accumulator directly -- no staging eviction.  RoPE: a-term
     cos multiply + final adds on DVE, halves-swapped sign-folded
     sin-multiplies on GPSIMD.
  3. q,k transposed to [d, tok] via PE transposes, software-pipelined
     two tiles back so the in-order PE stream never stalls on them.
  4. Per (head, 512-wide q chunk): scoresT = kT_tile^T @ qT (16 k-tiles,
     no max-subtraction: LN+rope bound |scores| ~ a few sigma), exp on
     ScalarE (PSUM->SBUF bf16, scale=1/sqrt(d)), AV split into M=65/M=64
     matmuls with a ones-column appended to v so row 64 of the first
     accumulator IS the softmax sum (no separate sums pass).
  5. Normalize: reciprocal straight from PSUM -> rank-1 ones matmul
     broadcast -> two half-height multiplies -> avn bf16.
  6. Out-proj per tok tile accumulating both heads, DVE evict to fp16,
     store (fp16 halves the store DMA; host upcasts and sums partials).

Scheduling: engine sequencers are IN-ORDER, so emission order is the
schedule.  Stage 2 is software-pipelined: block i's scores/AV loop
carries block i-1's normalize (injected mid-loop) and the out-proj of
the chunk completed one block ago.  All PSUM lives in ONE pool with
shared tags (A: qkv|scores|proj 3 bufs, B: v-psum|av1 2, C: tp|av2 2,
Dd: bcast 1 = exactly 8 banks) -- pool open/close would serialize the
stage transition on the allocator's release dependencies.

`repeat=k` wraps the body in a tc.For_i hardware loop so one NEFF
executes the whole kernel k times back-to-back; test.py uses the
slope between two repeat counts to measure true on-device time
through the high-latency PJRT tunnel.
"""

import math
import os
from contextlib import ExitStack

import numpy as np
import ml_dtypes

import concourse.bass as bass
import concourse.tile as tile
from concourse import bacc, mybir
from concourse.bass import ts, ds
from concourse.bass_utils import run_bass_kernel_spmd
from concourse.masks import make_identity

F32 = mybir.dt.float32
F16 = mybir.dt.float16
BF16 = mybir.dt.bfloat16

DIM = 1024
HEADS = 8
D = 128  # head dim
B = 2
N = 2048
EPS = 1e-6
HPC = 2  # heads per core
N_CORES = 8
P = 128  # partitions
QC = 512  # q chunk for attention
N_TILES = N // P  # 16
K_IN = DIM // P  # 8 k-tiles over input dim
W_OUT = HPC * 3 * D  # 768 qkv outdims per core
SCALE = 1.0 / math.sqrt(D)
# wqkv block layout (free offsets): q0,k0,q1,k1 then v0,v1
OFF_Q = [0 * D, 2 * D]
OFF_K = [1 * D, 3 * D]
OFF_V = [4 * D, 5 * D]


def build_core_graph(nc, n_tok=N, dtype_mm=BF16, emit_qk_bias=False,
                     shared_rope=True, repeat=1):
    """Emit the per-core program. All cores run the same graph (SPMD)."""
    n_tiles = n_tok // P
    n_qc = n_tok // QC if n_tok >= QC else 1
    qc = min(QC, n_tok)
    tpq = qc // P  # tok tiles per q chunk
    n_blk = min(4, n_tiles)  # token blocks for the load pipeline
    tpb = n_tiles // n_blk   # tiles per block
    RW = 2 * D if shared_rope else 4 * D  # rope row width (cos|sin[|cos|sin])

    # ---- dram parameters ----
    xT = nc.dram_tensor("xT", [DIM, n_tok], dtype_mm, kind="ExternalInput").ap()
    wqkv = nc.dram_tensor("wqkv", [DIM, W_OUT], dtype_mm, kind="ExternalInput").ap()
    wout = nc.dram_tensor("wout", [HPC * D, DIM], dtype_mm, kind="ExternalInput").ap()
    rope = nc.dram_tensor("rope", [n_tok, RW], BF16, kind="ExternalInput").ap()
    bqkv = None
    if emit_qk_bias:
        bqkv = nc.dram_tensor("bqkv", [1, W_OUT], F32, kind="ExternalInput").ap()
    out = nc.dram_tensor("out", [n_tok, DIM], F16, kind="ExternalOutput").ap()

    with tile.TileContext(nc) as tc, ExitStack() as ctx:
        const = ctx.enter_context(tc.tile_pool(name="const", bufs=1))
        big = ctx.enter_context(tc.tile_pool(name="big", bufs=1))

        # resident SBUF tensors
        xT_sb = big.tile([P, K_IN, n_tok], dtype_mm, tag="xT_sb")
        wqkv_sb = big.tile([P, K_IN, W_OUT], dtype_mm, tag="wqkv_sb")
        wout_sb = big.tile([P, HPC, DIM], dtype_mm, tag="wout_sb")
        rope_sb = big.tile([P, n_tiles, RW], BF16, tag="rope_sb")
        qT_all = big.tile([P, HPC, n_tok], dtype_mm, tag="qT_all")
        kT_all = big.tile([P, HPC, n_tok], dtype_mm, tag="kT_all")
        # v + trailing ones column: av1 takes cols [64:129] (v hi-half + ones
        # -> its psum row 64 IS the softmax sum), av2 takes cols [0:64]
        v_all = big.tile([P, n_tiles, HPC, D + 1], dtype_mm, tag="v_all")
        avn_sb = [big.tile([P, qc], dtype_mm, tag=f"avn{h}", name=f"avn{h}", bufs=min(2, n_qc))
                  for h in range(HPC)]

        ones_row = const.tile([1, P], F32)  # bcast rank-1 lhsT
        eps_col = const.tile([P, 1], F32)
        ident = const.tile([P, P], dtype_mm)
        bias_sb = const.tile([1, W_OUT], F32) if emit_qk_bias else None

        # single PSUM pool, tags shared across stages (8 banks, no barriers):
        #   A bufs=4: ps_a | sc | po     B bufs=2: ps_b | av1
        #   C bufs=2: tp | av2
        ps = ctx.enter_context(tc.tile_pool(name="ps", bufs=1, space="PSUM"))
        s1 = ctx.enter_context(tc.tile_pool(name="s1", bufs=6))
        s1small = ctx.enter_context(tc.tile_pool(name="s1small", bufs=8))
        probs = ctx.enter_context(tc.tile_pool(name="probs", bufs=6))
        s2 = ctx.enter_context(tc.tile_pool(name="s2", bufs=4))
        s3 = ctx.enter_context(tc.tile_pool(name="s3", bufs=8))

        def body():
            nc.vector.memset(ones_row[:], 1.0)
            nc.vector.memset(v_all[:, :, :, D], 1.0)
            nc.vector.memset(eps_col[:], EPS)
            make_identity(nc, ident)

            # ---- input loads: ONE queue, few big DMAs, dependency order ----
            # (HWDGE descriptor generation is ~600ns/DMA: batching transfers
            # matters as much as ordering them.)
            xT_r = xT.rearrange("(k p) n -> p k n", p=P)      # [P, K_IN, n_tok]
            rope_r = rope.rearrange("(t p) f -> p t f", p=P)  # [P, n_tiles, RW]
            # block 0 fine-grained so the first QKV matmul starts after ~220KB
            for kk in range(K_IN):
                nc.sync.dma_start(wqkv_sb[:, kk, :], wqkv[ts(kk, P), :])
                nc.sync.dma_start(xT_sb[:, kk, 0:tpb * P], xT[ts(kk, P), 0:tpb * P])
            if emit_qk_bias:
                nc.sync.dma_start(bias_sb[:], bqkv[:])
            nc.sync.dma_start(rope_sb[:, 0:tpb, :], rope_r[:, 0:tpb, :])
            for b in range(1, n_blk):
                tsl = ds(b * tpb * P, tpb * P)
                nc.sync.dma_start(xT_sb[:, :, tsl], xT_r[:, :, tsl])
                nc.sync.dma_start(rope_sb[:, b * tpb:(b + 1) * tpb, :],
                                  rope_r[:, b * tpb:(b + 1) * tpb, :])
            nc.sync.dma_start(wout_sb[:], wout.rearrange("(h p) w -> p h w", p=P))

            # ---------- stage 1: QKV + LN + RoPE ----------
            ro_tiles = {}  # (t, "q"/"k") -> rope-applied bf16 tile awaiting transpose

            def emit_transposes(t):
                ro = ro_tiles.pop(t)  # [P, 2(qk), HPC*D]
                for gi, dst in ((0, qT_all), (1, kT_all)):
                    for h in range(HPC):
                        tp = ps.tile([P, P], BF16, tag="C", bufs=2,
                                     name=f"tp{t}{gi}{h}")
                        nc.tensor.transpose(tp[:], ro[:, gi, ts(h, D)], ident[:])
                        # spread the psum->sbuf copies: q on ACT, k on DVE
                        if gi == 0:
                            nc.scalar.activation(dst[:, h, ts(t, P)], tp[:],
                                                 mybir.ActivationFunctionType.Copy)
                        else:
                            nc.vector.tensor_copy(dst[:, h, ts(t, P)], tp[:])

            for t in range(n_tiles):
                # two psum chunks: [q0,k0,q1,k1] (512) and [v0,v1] (256)
                ps_a = ps.tile([P, 4 * D], F32, tag="A", bufs=4, name=f"ps_a{t}")
                ps_b = ps.tile([P, 2 * D], F32, tag="B", bufs=2, name=f"ps_b{t}")
                for kk in range(K_IN):
                    lhsT = xT_sb[:, kk, ts(t, P)]
                    nc.tensor.matmul(ps_a[:], lhsT, wqkv_sb[:, kk, 0:4 * D],
                                     start=(kk == 0), stop=(kk == K_IN - 1))
                    nc.tensor.matmul(ps_b[:], lhsT, wqkv_sb[:, kk, 4 * D:W_OUT],
                                     start=(kk == 0), stop=(kk == K_IN - 1))
                if emit_qk_bias:
                    nc.tensor.matmul(ps_a[:], ones_row[:, :], bias_sb[:, 0:4 * D],
                                     start=False, stop=True)
                    nc.tensor.matmul(ps_b[:], ones_row[:, :], bias_sb[:, 4 * D:W_OUT],
                                     start=False, stop=True)

                # LN stats straight from PSUM (DVE).  The HW verifier requires
                # exactly one 6-element group per BNStats instruction.
                stats = s1small.tile([P, 4, 6], F32, tag="stats")
                mv = s1small.tile([P, 4, 2], F32, tag="mv")
                rs = s1small.tile([P, 4], F32, tag="rs")
                pa4 = ps_a.rearrange("p (s x) -> p s x", x=D)
                for s in range(4):
                    nc.vector.bn_stats(stats[:, s, :], pa4[:, s, :])
                    nc.vector.bn_aggr(mv[:, s, :], stats[:, s, :])
                # v evicted in one strided copy (ones column at D untouched)
                nc.scalar.activation(
                    v_all[:, t, :, 0:D],
                    ps_b.rearrange("p (h x) -> p h x", x=D),
                    mybir.ActivationFunctionType.Copy)

                # rs = 1/sqrt(var+eps)
                sd = s1small.tile([P, 4], F32, tag="sd")
                nc.scalar.activation(sd[:, :], mv[:, :, 1],
                                     mybir.ActivationFunctionType.Sqrt,
                                     bias=eps_col[:])
                nc.vector.reciprocal(rs[:, :], sd[:, :])

                # bias = -(mu*rs) so ScalarE computes (x*rs + bias) = (x-mu)*rs
                negrs = s1small.tile([P, 4], F32, tag="negrs")
                nc.gpsimd.tensor_scalar(negrs[:, :], rs[:, :], -1.0, None,
                                        mybir.AluOpType.mult)
                negmurs = s1small.tile([P, 4], F32, tag="negmurs")
                nc.gpsimd.tensor_tensor(negmurs[:, :], mv[:, :, 0], negrs[:, :],
                                        mybir.AluOpType.mult)
                # LN apply straight from PSUM (ACT) into one [P, 2(qk), 2h*D]
                # tile so rope processes q and k in single wide ops
                qkn = s1.tile([P, 2, HPC * D], BF16, tag="qkn")
                for h in range(HPC):
                    nc.scalar.activation(qkn[:, 0, ts(h, D)], ps_a[:, ts(2 * h, D)],
                                         mybir.ActivationFunctionType.Identity,
                                         bias=negmurs[:, 2 * h:2 * h + 1],
                                         scale=rs[:, 2 * h:2 * h + 1])
                    nc.scalar.activation(qkn[:, 1, ts(h, D)], ps_a[:, ts(2 * h + 1, D)],
                                         mybir.ActivationFunctionType.Identity,
                                         bias=negmurs[:, 2 * h + 1:2 * h + 2],
                                         scale=rs[:, 2 * h + 1:2 * h + 2])

                # rope over q and k at once: x4 [P, 2(qk), 2(h), D]; tables
                # broadcast over the head dim (and the qk dim when shared).
                # a = x*cos (DVE), b = halves-swapped x * sign-folded sin
                # (Pool), ro = a + b (DVE).
                x4 = qkn.rearrange("p g (s x) -> p g s x", x=D)
                if shared_rope:
                    ct = rope_sb[:, t, :].rearrange("p (o u x) -> p o u x", o=1, u=1)
                    cosT, sinT = ct[:, :, :, 0:D], ct[:, :, :, D:2 * D]
                else:
                    ct = rope_sb[:, t, :].rearrange("p (g u x) -> p g u x", g=2, u=1)
                    cosT, sinT = ct[:, :, :, 0:D], ct[:, :, :, D:2 * D]
                a = s1.tile([P, 2, HPC * D], BF16, tag="ra")
                a4 = a.rearrange("p g (s x) -> p g s x", x=D)
                bb = s1.tile([P, 2, HPC * D], BF16, tag="rb")
                b4 = bb.rearrange("p g (s x) -> p g s x", x=D)
                i0, i1 = bass.broadcast_tensor_aps(x4[:], cosT)
                nc.vector.tensor_tensor(a4[:], i0, i1, mybir.AluOpType.mult)
                j0, j1 = bass.broadcast_tensor_aps(x4[:, :, :, D // 2:D],
                                                   sinT[:, :, :, 0:D // 2])
                nc.gpsimd.tensor_tensor(b4[:, :, :, 0:D // 2], j0, j1,
                                        mybir.AluOpType.mult)
                j0, j1 = bass.broadcast_tensor_aps(x4[:, :, :, 0:D // 2],
                                                   sinT[:, :, :, D // 2:D])
                nc.gpsimd.tensor_tensor(b4[:, :, :, D // 2:D], j0, j1,
                                        mybir.AluOpType.mult)
                ro = s1.tile([P, 2, HPC * D], dtype_mm, tag="ro")
                nc.vector.tensor_tensor(ro[:], a[:], bb[:], mybir.AluOpType.add)
                ro_tiles[t] = ro

                # PE transposes, software-pipelined 3 tiles back
                if t >= 3:
                    emit_transposes(t - 3)
            for tt in range(max(0, n_tiles - 3), n_tiles):
                emit_transposes(tt)

            # ---------- stage 2+3: attention + out-projection, sw-pipelined ----------
            av_tiles = {}

            def emit_block(qi, h, mid=None, work=()):
                """scores + exp + fused AV/sums over all k tiles for (qi, h).
                `mid` fires once mid-loop; `work` items (prev chunk's proj
                pieces) are spread one per k tile from kt=2 on, so their
                psum-bank grabs interleave with the score tiles'."""
                av1 = ps.tile([D // 2 + 1, qc], F32, tag="B", bufs=2,
                              name=f"av1_{qi}_{h}")
                av2 = ps.tile([D // 2, qc], F32, tag="C", bufs=2,
                              name=f"av2_{qi}_{h}")
                # `work` items read the avn generation that `mid` (the next
                # normalize) overwrites -- every item MUST be emitted before
                # mid() or it binds to the wrong generation.
                mid_kt = min(12, n_tiles - 1)
                work = list(work)
                for kt in range(n_tiles):
                    if kt == mid_kt:
                        for w in work:
                            w()
                        work = []
                        if mid is not None:
                            mid()
                    if kt >= 2 and work:
                        work.pop(0)()
                    sc = ps.tile([P, qc], F32, tag="A", bufs=4, name=f"sc{qi}{h}{kt}")
                    nc.tensor.matmul(sc[:], kT_all[:, h, ts(kt, P)],
                                     qT_all[:, h, ds(qi * qc, qc)],
                                     start=True, stop=True)
                    pr = probs.tile([P, qc], dtype_mm, tag="pr")
                    nc.scalar.activation(pr[:], sc[:],
                                         mybir.ActivationFunctionType.Exp,
                                         scale=SCALE)
                    nc.tensor.matmul(av1[:], v_all[:, kt, h, D // 2:D + 1], pr[:],
                                     start=(kt == 0), stop=(kt == n_tiles - 1))
                    nc.tensor.matmul(av2[:], v_all[:, kt, h, 0:D // 2], pr[:],
                                     start=(kt == 0), stop=(kt == n_tiles - 1))
                av_tiles[(qi, h)] = (av1, av2)

            def emit_recip_bc(qi, h):
                """recip of the sums row (av1 row 64) + partition broadcast on
                GPSIMD (idle in this stage) -> bc_sb, no psum bank needed."""
                av1, av2 = av_tiles[(qi, h)]
                rcp = s2.tile([1, qc], F32, tag="rcp", name=f"rcp{qi}{h}")
                nc.vector.reciprocal(rcp[:], av1[D // 2:D // 2 + 1, :])
                bc_sb = s2.tile([P, qc], F32, tag="bc_sb", name=f"bc{qi}{h}")
                nc.gpsimd.partition_broadcast(bc_sb[:], rcp[:])
                av_tiles[(qi, h)] = (av1, av2, bc_sb)

            def emit_normalize(qi, h):
                """scale both AV halves -> avn (av1 rows 0:64 are v dims
                64:128, av2 rows are v dims 0:64)."""
                av1, av2, bc_sb = av_tiles.pop((qi, h))
                nc.vector.tensor_tensor(avn_sb[h][0:D // 2, :], av2[:, :],
                                        bc_sb[0:D // 2, :], mybir.AluOpType.mult)
                nc.vector.tensor_tensor(avn_sb[h][D // 2:D, :], av1[0:D // 2, :],
                                        bc_sb[D // 2:D, :], mybir.AluOpType.mult)

            def proj_chunk(qi, ti, c, evict_act=False):
                t = qi * tpq + ti
                po = ps.tile([P, QC], F32, tag="A", bufs=4, name=f"po{t}{c}")
                for h in range(HPC):
                    nc.tensor.matmul(po[:], avn_sb[h][:, ts(ti, P)],
                                     wout_sb[:, h, ts(c, QC)],
                                     start=(h == 0), stop=(h == HPC - 1))
                ot = s3.tile([P, QC], F16, tag="ot")
                if evict_act:
                    nc.scalar.activation(ot[:], po[:],
                                         mybir.ActivationFunctionType.Copy)
                else:
                    nc.vector.tensor_copy(ot[:], po[:])
                nc.scalar.dma_start(out[ts(t, P), ts(c, QC)], ot[:])

            def proj_work(qi):
                return [lambda ti=ti, c=c: proj_chunk(qi, ti, c)
                        for ti in range(tpq) for c in range(DIM // QC)]

            blocks = [(qi, h) for qi in range(n_qc) for h in range(HPC)]
            for i, (qi, h) in enumerate(blocks):
                mid = (lambda p=blocks[i - 1]: emit_normalize(*p)) if i >= 1 else None
                # proj of chunk qi-1 rides inside block (qi, 1): its avn was
                # completed by the normalize injected into block (qi, 0)
                work = proj_work(qi - 1) if (h == HPC - 1 and qi >= 1) else ()
                emit_block(qi, h, mid=mid, work=work)
                emit_recip_bc(qi, h)
            emit_normalize(*blocks[-1])
            for w in proj_work(n_qc - 1):
                w()

        if repeat == 1:
            body()
        else:
            with tc.For_i(0, repeat, 1):
                body()

    return nc


# ---------------- host side ----------------

def _rope_tables(q_gamma, k_gamma, rope_cos, rope_sin, n_tok=N):
    """Build the on-device rope table(s): [cos|sin] rows, sign of the
    rotate-half folded into sin, gamma folded in, NO head duplication and
    NO score scale (the exp activation applies 1/sqrt(d)).  Returns
    (table [n_tok, 2D or 4D], shared: bool)."""
    bf = ml_dtypes.bfloat16
    perm = np.concatenate([np.arange(0, D, 2), np.arange(1, D, 2)])
    partner = np.concatenate([np.arange(0, D, 2) + 1, np.arange(1, D, 2) - 1])
    sgn = np.concatenate([-np.ones(D // 2, np.float32), np.ones(D // 2, np.float32)])
    cosP = rope_cos[:n_tok, perm]
    sinP = rope_sin[:n_tok, perm]

    def tab(g):
        c = cosP * g[perm][None, :]
        s = (sinP * g[partner][None, :]) * sgn[None, :]
        return np.concatenate([c, s], axis=1)

    shared = bool(np.allclose(q_gamma, k_gamma))
    if shared:
        t = tab(q_gamma)
    else:
        t = np.concatenate([tab(q_gamma), tab(k_gamma)], axis=1)
    return np.ascontiguousarray(t.astype(bf)), shared


def _prep_core_inputs(x, Wqkv_w, Wqkv_b, q_gamma, k_gamma, out_w,
                      rope_cos, rope_sin, n_tok=N):
    """Build the 8 per-core input dicts (numpy, host-side sharding)."""
    bf = ml_dtypes.bfloat16
    # even-first permutation of head_dim (rotate_half becomes a 64-half swap)
    perm = np.concatenate([np.arange(0, D, 2), np.arange(1, D, 2)])
    rope_tab, shared = _rope_tables(q_gamma, k_gamma, rope_cos, rope_sin, n_tok)

    Wr = Wqkv_w.reshape(3, HEADS, D, DIM)
    in_maps = []
    for c in range(N_CORES):
        b = c // 4
        hs = [2 * (c % 4), 2 * (c % 4) + 1]
        xT = np.ascontiguousarray(x[b, :n_tok].T).astype(bf)
        blocks = []
        for h in hs:
            blocks.append(Wr[0, h][perm].T)  # q, dim-permuted  [DIM,128]
            blocks.append(Wr[1, h][perm].T)  # k, dim-permuted
        for h in hs:
            blocks.append(Wr[2, h].T)        # v, natural
        wqkv = np.concatenate(blocks, axis=1).astype(bf)  # [DIM, 768]
        wout = np.concatenate(
            [out_w[:, h * D:(h + 1) * D].T for h in hs], axis=0).astype(bf)  # [256,DIM]
        in_maps.append({
            "xT": xT,
            "wqkv": np.ascontiguousarray(wqkv),
            "wout": np.ascontiguousarray(wout),
            "rope": rope_tab,
        })
    return in_maps, shared


def kernel(x, Wqkv_w, Wqkv_b, q_gamma, q_beta, k_gamma, k_beta,
           out_w, out_b, rope_cos, rope_sin, trace=False, tmpdir=None):
    x = np.asarray(x, np.float32)
    Wqkv_w = np.asarray(Wqkv_w, np.float32)
    Wqkv_b = np.asarray(Wqkv_b, np.float32)
    q_gamma = np.asarray(q_gamma, np.float32)
    q_beta = np.asarray(q_beta, np.float32)
    k_gamma = np.asarray(k_gamma, np.float32)
    k_beta = np.asarray(k_beta, np.float32)
    out_w = np.asarray(out_w, np.float32)
    out_b = np.asarray(out_b, np.float32)
    rope_cos = np.asarray(rope_cos, np.float32)
    rope_sin = np.asarray(rope_sin, np.float32)

    assert np.allclose(q_beta, 0) and np.allclose(k_beta, 0), \
        "nonzero q/k layernorm beta not supported by this kernel build"
    emit_qk_bias = not (np.allclose(Wqkv_b[:DIM], 0) and np.allclose(Wqkv_b[DIM:2 * DIM], 0))

    in_maps, shared = _prep_core_inputs(x, Wqkv_w, Wqkv_b, q_gamma, k_gamma,
                                        out_w, rope_cos, rope_sin)

    nc = bacc.Bacc("TRN2", target_bir_lowering=False, debug=False,
                   num_devices=N_CORES)
    build_core_graph(nc, n_tok=N, emit_qk_bias=emit_qk_bias, shared_rope=shared)
    nc.compile()

    if emit_qk_bias:
        for c in range(N_CORES):
            hs = [2 * (c % 4), 2 * (c % 4) + 1]
            bq = Wqkv_b[:DIM].reshape(HEADS, D)
            bk = Wqkv_b[DIM:2 * DIM].reshape(HEADS, D)
            perm = np.concatenate([np.arange(0, D, 2), np.arange(1, D, 2)])
            blocks = [np.zeros(0, np.float32)]
            for h in hs:
                blocks += [bq[h][perm], bk[h][perm]]
            blocks += [np.zeros(2 * D, np.float32)]
            in_maps[c]["bqkv"] = np.concatenate(blocks)[None, :].astype(np.float32)

    res = run_bass_kernel_spmd(nc, in_maps, core_ids=list(range(N_CORES)),
                               trace=trace, tmpdir=tmpdir)
    partials = [np.asarray(r["out"], np.float32) for r in res.results]

    # host gather: sum the 4 head-group partials per batch; fold v-bias + out_b
    bv = Wqkv_b[2 * DIM:]
    bias_row = out_b + bv @ out_w.T  # [DIM]
    outp = np.empty((B, N, DIM), np.float32)
    for b in range(B):
        outp[b] = sum(partials[4 * b:4 * b + 4]) + bias_row[None, :]
    kernel.last_exec_time_ns = res.exec_time_ns
    return outp
